# revision 1
# baseline (speedup 1.0000x reference)
"""MultiHeadLinearAttention Trainium2 kernel (8-core SPMD, fp32r matmuls).

Sharding: 16384 tokens split across 8 cores (core c: batch c//2, sequence half
c%2). All projections/attention/out-proj are local; the only cross-core
dependency is the per-batch KV summary (kv [H,DK,DK] + ksum [D]) reduced via a
266KB pair-wise AllReduce, overlapped with the boundary weight loads.

Layouts (no transposes anywhere on device):
  - host pre-transposes x (feature-major xT [D,T]) and weights (wT [din,dout])
  - q GLU computed feature-major (bias per-partition via ACT/stt)
  - k/v GLU computed token-major (bias via K=1 ones-outer matmul into PSUM)
  - kv/ksum contraction over tokens; single PSUM accumulation group per bank
    (start only on the globally-first matmul -- has_written is per element)
  - z via block-diag ksum lhsT; 1/(z+eps) broadcast across partitions via a
    K=1 outer-product, applied at num-eviction (fused DVE multiply)
  - out-proj consumes feature-major attn directly

Pipelining: PE executes in order, so reduction matmuls that depend on
DVE/ACT/GPSIMD-produced tiles are emitted one iteration late (ksum/kv by one
token tile, the attention tail by one chunk) to keep the PE queue from
head-of-line blocking on elementwise chains.
"""
import os
from contextlib import ExitStack

import ml_dtypes
import numpy as np
import concourse.mybir as mybir
import concourse.tile as tile
from concourse import bacc
from concourse.bass_utils import run_bass_kernel_spmd

F32 = mybir.dt.float32
F32R = mybir.dt.float32r
ACTF = mybir.ActivationFunctionType
ALU = mybir.AluOpType

B, S, D, H = 4, 4096, 1024, 16
DK = D // H          # 64
EPS = 1e-6
NCORES = 8
T = B * S // NCORES  # 2048 tokens per core
P = 128
NM = T // P          # 16 token tiles
NCD = D // P         # 8 d-chunks
CH = 256             # stage-2 token chunk
NCH = T // CH        # 8 chunks
GROUPS = [[0, 1], [2, 3], [4, 5], [6, 7]]


def build(single_core=False, stages="12"):
    nc = bacc.Bacc("TRN2", target_bir_lowering=False, debug=False,
                   num_devices=1 if single_core else NCORES)
    dt_in = {}

    def inp(name, shape):
        dt_in[name] = nc.dram_tensor(name, shape, F32, kind="ExternalInput").ap()

    for name, shape in (
        ("xqT", [D, T]), ("xkT", [D, T]), ("xvT", [D, T]),
        ("wq1T", [D, D]), ("wq2T", [D, D]), ("wk1T", [D, D]), ("wk2T", [D, D]),
        ("wv1T", [D, D]), ("wv2T", [D, D]), ("woT", [D, D]),
        ("bq1c", [P, NCD]), ("bq2c", [P, NCD]),
        ("bk1r", [P, D]), ("bk2r", [P, D]),
        ("bv1r", [P, D]), ("bv2r", [P, D]), ("bor", [P, D]),
        ("ones_row", [1, P]), ("zeros16", [P, H]), ("maskp", [P, NM]),
    ):
        inp(name, shape)
    dt_in["ones_col_bf"] = nc.dram_tensor("ones_col_bf", [P, 1], mybir.dt.bfloat16,
                                          kind="ExternalInput").ap()
    out = nc.dram_tensor("out", [T, D], F32, kind="ExternalOutput").ap()

    with tile.TileContext(nc) as tc:
        _emit(nc, tc, dt_in, out, single_core, stages)
    nc.compile()
    return nc


def _emit(nc, tc, dt, out, single_core, stages="12"):
    def mm(psum, lhsT, rhs, start, stop):
        nc.tensor.matmul(psum, lhsT, rhs, start=start, stop=stop)

    with ExitStack() as st0:
        const = st0.enter_context(tc.tile_pool(name="const", bufs=1))
        dram = st0.enter_context(tc.tile_pool(name="dram", bufs=1, space="DRAM"))
        kvres = st0.enter_context(tc.tile_pool(name="kvres", bufs=1))
        kvstage_ctx = ExitStack()
        kvstage = kvstage_ctx.enter_context(tc.tile_pool(name="kvstage", bufs=1))

        ones_sb = const.tile([1, P], F32R, tag="ones", name="ones")
        nc.sync.dma_start(ones_sb[:], dt["ones_row"][:].bitcast(F32R))
        bcol = {}
        for nm in ("bq1", "bq2"):
            bcol[nm] = const.tile([P, NCD], F32, tag=f"col_{nm}", name=f"col_{nm}")
            nc.sync.dma_start(bcol[nm][:], dt[nm + "c"][:])

        def bias_rep(pool, nm):
            t = pool.tile([P, D], F32, tag=f"rep_{nm}", name=f"rep_{nm}")
            nc.sync.dma_start(t[:], dt[nm][:])
            return t

        has1 = "1" in stages
        has2 = "2" in stages
        if not has1:
            kv_acc = [kvstage.tile([64, 512], F32, tag=f"kv_acc{i}", name=f"kv_acc{i}")
                      for i in range(2)]
            for i in range(2):
                nc.any.memset(kv_acc[i][:], 1.0)
            cc_ks_sb = kvstage.tile([1, D], F32, tag="cc_ks_sb", name="cc_ks_sb")
            nc.any.memset(cc_ks_sb[:], 1.0)

        st1 = st0.enter_context(ExitStack())
        phik_pool = st1.enter_context(tc.tile_pool(name="phik", bufs=1))
        phi_k = [phik_pool.tile([P, D], mybir.dt.bfloat16, tag=f"phik_{m}",
                                name=f"phik_{m}")
                 for m in range(NM)] if has1 else []

        # ================= stage 1a: k projection -> phi_k, ksum =================
        with ExitStack() as st1a:
            wkp = st1a.enter_context(tc.tile_pool(name="wk", bufs=1))
            xkp = st1a.enter_context(tc.tile_pool(name="xk", bufs=2))
            t1a = st1a.enter_context(tc.tile_pool(name="t1a", bufs=2))
            pk1p = st1a.enter_context(tc.tile_pool(name="pk1", bufs=2, space="PSUM"))
            pk2p = st1a.enter_context(tc.tile_pool(name="pk2", bufs=2, space="PSUM"))
            pksp = st1a.enter_context(tc.tile_pool(name="pks", bufs=1, space="PSUM"))
            wk_sb = {}
            for w, src in (("w1", "wk1T"), ("w2", "wk2T")):
                for c in range(NCD):
                    wk_sb[w, c] = wkp.tile([P, D], F32R, tag=f"wk_{w}_{c}",
                                           name=f"wk_{w}_{c}")
                    nc.scalar.dma_start(wk_sb[w, c][:],
                                      dt[src][c * P:(c + 1) * P, :].bitcast(F32R))
            psum_ks = [pksp.tile([1, 512], F32, tag=f"ks{i}", name=f"ks{i}")
                       for i in range(2)]
            ones_col = wkp.tile([P, 1], mybir.dt.bfloat16, tag="ones_col",
                                name="ones_col")
            nc.gpsimd.dma_start(ones_col[:], dt["ones_col_bf"][:])
            mask_sb = wkp.tile([P, NM], F32, tag="mask", name="mask")
            nc.gpsimd.dma_start(mask_sb[:], dt["maskp"][:])
            brep_k = {nm: bias_rep(wkp, nm) for nm in ("bk1r", "bk2r")}

            def ksum_tail(m):
                for i in range(2):
                    mm(psum_ks[i][:], ones_col[:],
                       phi_k[m][:, i * 512:(i + 1) * 512],
                       start=(m == 0), stop=(m == NM - 1))

            for m in range(NM if has1 else 0):
                xk_m = xkp.tile([P, D], F32R, tag="xk", name="xk")
                for c in range(NCD):
                    nc.sync.dma_start(
                        xk_m[:, c * P:(c + 1) * P],
                        dt["xkT"][c * P:(c + 1) * P, m * P:(m + 1) * P].bitcast(F32R))
                kgs, tmins = [], []
                for n in range(2):
                    ns = slice(n * 512, (n + 1) * 512)
                    p1 = pk1p.tile([P, 512], F32, tag="pk1", name="pk1")
                    p2 = pk2p.tile([P, 512], F32, tag="pk2", name="pk2")
                    for c in range(NCD):
                        mm(p1[:], xk_m[:, c * P:(c + 1) * P], wk_sb["w1", c][:, ns],
                           start=(c == 0), stop=(c == NCD - 1))
                    for c in range(NCD):
                        mm(p2[:], xk_m[:, c * P:(c + 1) * P], wk_sb["w2", c][:, ns],
                           start=(c == 0), stop=(c == NCD - 1))
                    t1 = t1a.tile([P, 512], F32, tag="t1", name="t1")
                    nc.vector.tensor_tensor(t1[:], p1[:], brep_k["bk1r"][:, ns],
                                            ALU.add)
                    a1 = t1a.tile([P, 512], F32, tag="a1", name="a1")
                    nc.scalar.activation(a1[:], t1[:], ACTF.Sigmoid)
                    g1 = t1a.tile([P, 512], F32, tag="g1", name="g1")
                    nc.vector.tensor_tensor(g1[:], a1[:], t1[:], ALU.mult)
                    t2b = t1a.tile([P, 512], F32, tag="t2b", name="t2b")
                    nc.vector.tensor_tensor(t2b[:], p2[:], brep_k["bk2r"][:, ns],
                                            ALU.add)
                    kg = t1a.tile([P, 512], F32, tag="kg", name="kg", bufs=2)
                    nc.vector.tensor_tensor(kg[:], g1[:], t2b[:], ALU.mult)
                    tmin = t1a.tile([P, 512], F32, tag="tmin", name="tmin", bufs=2)
                    nc.vector.tensor_scalar_min(tmin[:], kg[:], 0.0)
                    kgs.append(kg)
                    tmins.append(tmin)
                for n in range(2):  # Exp batch + phi assembly
                    ns = slice(n * 512, (n + 1) * 512)
                    texp = t1a.tile([P, 512], F32, tag="texp", name="texp")
                    nc.scalar.activation(texp[:], tmins[n][:], ACTF.Exp)
                    trel = t1a.tile([P, 512], F32, tag="trel", name="trel")
                    nc.vector.tensor_scalar(trel[:], kgs[n][:], 0.0,
                                            mask_sb[:, m:m + 1], ALU.max, ALU.mult)
                    # phi_k = exp(min(kg,0))*mask + relu(kg)*mask
                    nc.vector.scalar_tensor_tensor(
                        phi_k[m][:, ns], texp[:], mask_sb[:, m:m + 1], trel[:],
                        ALU.mult, ALU.add)
                if m >= 2:
                    ksum_tail(m - 2)
            if has1:
                ksum_tail(NM - 2)
                ksum_tail(NM - 1)
                cc_ks_sb = kvstage.tile([1, D], F32, tag="cc_ks_sb", name="cc_ks_sb")
                for i in range(2):
                    nc.vector.tensor_copy(cc_ks_sb[0:1, i * 512:(i + 1) * 512],
                                          psum_ks[i][:])

        # wq w1 prefetch: space freed by wk pool (stage 1a) fits half of wq,
        # letting its DMA overlap stage-1b compute instead of the boundary
        stw = st0.enter_context(ExitStack())
        wqp = stw.enter_context(tc.tile_pool(name="wq1p", bufs=1, side="right"))
        wq_sb = {}
        for c in range(NCD):
            wq_sb["w1", c] = wqp.tile([P, D], F32R, tag=f"wq_w1_{c}",
                                      name=f"wq_w1_{c}")
            nc.scalar.dma_start(wq_sb["w1", c][:],
                                dt["wq1T"][c * P:(c + 1) * P, :].bitcast(F32R))

        # ============== stage 1b: v projection + kv accumulation ==============
        with ExitStack() as st1b:
            wvp = st1b.enter_context(tc.tile_pool(name="wv", bufs=1))
            xvp = st1b.enter_context(tc.tile_pool(name="xv", bufs=3))
            t1b = st1b.enter_context(tc.tile_pool(name="t1b", bufs=3))
            vgp = st1b.enter_context(tc.tile_pool(name="vgp", bufs=3))
            pv1p = st1b.enter_context(tc.tile_pool(name="pv1", bufs=3, space="PSUM"))
            pv2p = st1b.enter_context(tc.tile_pool(name="pv2", bufs=3, space="PSUM"))
            pkvp = st1b.enter_context(tc.tile_pool(name="pkv", bufs=1, space="PSUM"))
            wv_sb = {}
            for w, src in (("w1", "wv1T"), ("w2", "wv2T")):
                for c in range(NCD):
                    wv_sb[w, c] = wvp.tile([P, D], F32R, tag=f"wv_{w}_{c}",
                                           name=f"wv_{w}_{c}")
                    nc.scalar.dma_start(wv_sb[w, c][:],
                                      dt[src][c * P:(c + 1) * P, :].bitcast(F32R))
            brep_v = {nm: bias_rep(wvp, nm) for nm in ("bv1r", "bv2r")}
            if has1:
                psum_kv = [pkvp.tile([64, 512], F32, tag=f"pkv{i}", name=f"pkv{i}")
                           for i in range(2)]

            def kv_tail(m, vg_m):
                # one global accumulation group per bank: start only on the very
                # first matmul (has_written is per element; first write of each
                # element overwrites, later ones accumulate)
                for h in range(H):
                    hs = slice(h * DK, (h + 1) * DK)
                    first = (m == 0 and h % 8 == 0)
                    last = (m == NM - 1 and h % 8 == 7)
                    nc.tensor.matmul(
                        psum_kv[h // 8][0:64, (h % 8) * DK:(h % 8 + 1) * DK],
                        phi_k[m][:, hs], vg_m[:, hs],
                        start=first, stop=last,
                        skip_group_check=not (first or last))

            vg_hist = []
            for m in range(NM if has1 else 0):
                xv_m = xvp.tile([P, D], F32R, tag="xv", name="xv")
                for c in range(NCD):
                    nc.sync.dma_start(
                        xv_m[:, c * P:(c + 1) * P],
                        dt["xvT"][c * P:(c + 1) * P, m * P:(m + 1) * P].bitcast(F32R))
                vg = vgp.tile([P, D], mybir.dt.bfloat16, tag="vg", name="vg")
                for n in range(2):
                    ns = slice(n * 512, (n + 1) * 512)
                    p1 = pv1p.tile([P, 512], F32, tag="pv1", name="pv1")
                    p2 = pv2p.tile([P, 512], F32, tag="pv2", name="pv2")
                    for c in range(NCD):
                        mm(p1[:], xv_m[:, c * P:(c + 1) * P], wv_sb["w1", c][:, ns],
                           start=(c == 0), stop=(c == NCD - 1))
                    for c in range(NCD):
                        mm(p2[:], xv_m[:, c * P:(c + 1) * P], wv_sb["w2", c][:, ns],
                           start=(c == 0), stop=(c == NCD - 1))
                    t1 = t1b.tile([P, 512], F32, tag="vt1", name="vt1")
                    nc.vector.tensor_tensor(t1[:], p1[:], brep_v["bv1r"][:, ns],
                                            ALU.add)
                    a1 = t1b.tile([P, 512], F32, tag="va1", name="va1")
                    nc.scalar.activation(a1[:], t1[:], ACTF.Sigmoid)
                    g1 = t1b.tile([P, 512], F32, tag="vg1", name="vg1")
                    nc.vector.tensor_tensor(g1[:], a1[:], t1[:], ALU.mult)
                    t2b = t1b.tile([P, 512], F32, tag="vt2", name="vt2")
                    nc.vector.tensor_tensor(t2b[:], p2[:], brep_v["bv2r"][:, ns],
                                            ALU.add)
                    nc.vector.tensor_tensor(vg[:, ns], g1[:], t2b[:], ALU.mult)
                vg_hist.append(vg)
                if m >= 2:
                    kv_tail(m - 2, vg_hist[m - 2])
            if has1:
                kv_tail(NM - 2, vg_hist[NM - 2])
                kv_tail(NM - 1, vg_hist[NM - 1])
                kv_acc = [kvstage.tile([64, 512], F32, tag=f"kv_acc{i}",
                                       name=f"kv_acc{i}") for i in range(2)]
                for i in range(2):
                    nc.vector.tensor_copy(kv_acc[i][:], psum_kv[i][:])

        st1.close()  # frees phi_k SBUF before stage 2

        # ============ collective: pair AllReduce of kv + ksum ============
        cc_in = dram.tile([130, 512], F32)
        cc_out = dram.tile([130, 512], F32)
        nc.gpsimd.dma_start(cc_in[0:64, :], kv_acc[0][:])
        nc.gpsimd.dma_start(cc_in[64:128, :], kv_acc[1][:])
        nc.gpsimd.dma_start(cc_in[128:130, :], cc_ks_sb[:])
        kvstage_ctx.close()  # kv staging tiles no longer needed in SBUF
        if single_core:
            nc.sync.dma_start(cc_out[:], cc_in[:])
        else:
            nc.gpsimd.collective_compute(
                "AllReduce", ALU.add, replica_groups=GROUPS,
                ins=[cc_in.opt()], outs=[cc_out.opt()])

        # reduced kv -> pair-packed sbuf tile; ksum -> block-diag lhsT tiles
        kv_pairs = kvres.tile([P, 512], F32R, tag="kv_pairs", name="kv_pairs")
        for h in range(H):
            r0 = 0 if h < 8 else 64
            nc.gpsimd.dma_start(
                kv_pairs[(h % 2) * 64:(h % 2) * 64 + 64,
                         (h // 2) * DK:(h // 2 + 1) * DK],
                cc_out[r0:r0 + 64, (h % 8) * DK:(h % 8 + 1) * DK].bitcast(F32R))
        ksum_bd = []
        for c in range(NCD):
            bd = kvres.tile([P, H], F32R, tag=f"bd{c}", name=f"bd{c}")
            nc.gpsimd.dma_start(bd[:], dt["zeros16"][:].bitcast(F32R))
            # ksum[d] lives at cc_out[128 + d // 512, d % 512]
            for half, cs in ((0, 2 * c), (64, 2 * c + 1)):
                d0 = c * P + half
                nc.gpsimd.dma_start(
                    bd[half:half + 64, cs:cs + 1],
                    cc_out[128 + d0 // 512:129 + d0 // 512,
                           d0 % 512:d0 % 512 + 64].bitcast(F32R))
            ksum_bd.append(bd)

        # ============ stage 2: q -> phi_q -> z -> attn -> out ============
        with ExitStack() as st2:
            wop = st2.enter_context(tc.tile_pool(name="wo", bufs=1))
            xqp = st2.enter_context(tc.tile_pool(name="xq", bufs=2))
            phiqp = st2.enter_context(tc.tile_pool(name="phiq", bufs=2))
            attnp = st2.enter_context(tc.tile_pool(name="attn", bufs=2))
            t2 = st2.enter_context(tc.tile_pool(name="t2", bufs=3))
            tz = st2.enter_context(tc.tile_pool(name="tz", bufs=1))
            rrp = st2.enter_context(tc.tile_pool(name="rr", bufs=2))
            osbp = st2.enter_context(tc.tile_pool(name="osb", bufs=2))
            pq1p = st2.enter_context(tc.tile_pool(name="pq1", bufs=2, space="PSUM"))
            pq2p = st2.enter_context(tc.tile_pool(name="pq2", bufs=2, space="PSUM"))
            pzp = st2.enter_context(tc.tile_pool(name="pz", bufs=1, space="PSUM"))
            prp = st2.enter_context(tc.tile_pool(name="pr", bufs=1, space="PSUM"))
            pnp = st2.enter_context(tc.tile_pool(name="pn", bufs=1, space="PSUM"))
            pop = st2.enter_context(tc.tile_pool(name="po", bufs=1, space="PSUM"))
            wq2p = st2.enter_context(tc.tile_pool(name="wq2p", bufs=1))
            for c in range(NCD):
                wq_sb["w2", c] = wq2p.tile([P, D], F32R, tag=f"wq_w2_{c}",
                                           name=f"wq_w2_{c}")
                nc.scalar.dma_start(wq_sb["w2", c][:],
                                    dt["wq2T"][c * P:(c + 1) * P, :].bitcast(F32R))
            brep_o = bias_rep(wop, "bor")
            wo_sb = {}
            for c in range(NCD):
                wo_sb[c] = wop.tile([P, D], F32R, tag=f"wo_{c}", name=f"wo_{c}")
                nc.scalar.dma_start(wo_sb[c][:],
                                  dt["woT"][c * P:(c + 1) * P, :].bitcast(F32R))

            def tail_head(phi_q):
                # z -> r for a finished chunk; returns (r2 tiles, attn tiles)
                pz = pzp.tile([H, CH], F32, tag="pz", name="pz")
                for c in range(NCD):
                    mm(pz[:], ksum_bd[c][:], phi_q[c][:],
                       start=(c == 0), stop=(c == NCD - 1))
                zeps = tz.tile([H, CH], F32, tag="zeps", name="zeps")
                nc.vector.tensor_scalar_add(zeps[:], pz[:], EPS)
                r_sb = tz.tile([H, CH], F32, tag="r_sb", name="r_sb")
                nc.vector.reciprocal(r_sb[:], zeps[:])
                r2s = []
                for pair in range(NCD):
                    r2 = tz.tile([1, 2 * CH], F32R, tag="r2", name="r2", bufs=NCD)
                    nc.gpsimd.dma_start(r2[:],
                                        r_sb[2 * pair:2 * pair + 2, :].bitcast(F32R))
                    r2s.append(r2)
                attn = [attnp.tile([P, CH], F32R, tag=f"attn{c}", name=f"attn{c}")
                        for c in range(NCD)]
                return r2s, attn

            def tail_pair(phi_q, r2s, attn, pair):
                r2 = r2s[pair]
                for hb, h in ((0, 2 * pair), (64, 2 * pair + 1)):
                    pr = prp.tile([64, CH], F32, tag="pr", name="pr")
                    mm(pr[:], ones_sb[0:1, 0:64],
                       r2[0:1, (h % 2) * CH:((h % 2) + 1) * CH],
                       start=True, stop=True)
                    r_rep = rrp.tile([64, CH], F32, tag="r_rep", name="r_rep")
                    nc.vector.tensor_copy(r_rep[:], pr[:])
                    pn = pnp.tile([64, CH], F32, tag="pn", name="pn")
                    mm(pn[:], kv_pairs[hb:hb + 64, pair * DK:(pair + 1) * DK],
                       phi_q[pair][hb:hb + 64, :], start=True, stop=True)
                    nc.vector.tensor_tensor(attn[pair][hb:hb + 64, :],
                                            pn[:], r_rep[:], ALU.mult)

            def tail_out(ch, attn):
                for mt in range(CH // P):
                    o_sb = osbp.tile([P, D], F32, tag="o_sb", name="o_sb")
                    for n in range(2):
                        ns = slice(n * 512, (n + 1) * 512)
                        po = pop.tile([P, 512], F32, tag="po", name="po")
                        for c in range(NCD):
                            mm(po[:], attn[c][:, mt * P:(mt + 1) * P],
                               wo_sb[c][:, ns], start=(c == 0), stop=(c == NCD - 1))
                        nc.vector.tensor_tensor(o_sb[:, ns], po[:],
                                                brep_o[:, ns], ALU.add)
                    row0 = ch * CH + mt * P
                    nc.gpsimd.dma_start(out[row0:row0 + P, :], o_sb[:])

            pending = None
            for ch in range(NCH if has2 else 0):
                ts = slice(ch * CH, (ch + 1) * CH)
                xq_ch = xqp.tile([P, NCD * CH], F32R, tag="xq", name="xq")
                for c in range(NCD):
                    nc.sync.dma_start(
                        xq_ch[:, c * CH:(c + 1) * CH],
                        dt["xqT"][c * P:(c + 1) * P, ts].bitcast(F32R))
                phi_q = [phiqp.tile([P, CH], F32R, tag=f"phiq{mc}", name=f"phiq{mc}")
                         for mc in range(NCD)]
                if pending is not None:
                    p_ch, p_phi = pending
                    p_r2s, p_attn = tail_head(p_phi)
                qgs, qtmins = [], []
                for mc in range(NCD):
                    ms = slice(mc * P, (mc + 1) * P)
                    p1 = pq1p.tile([P, CH], F32, tag="pq1", name="pq1")
                    p2 = pq2p.tile([P, CH], F32, tag="pq2", name="pq2")
                    for c in range(NCD):
                        mm(p1[:], wq_sb["w1", c][:, ms],
                           xq_ch[:, c * CH:(c + 1) * CH],
                           start=(c == 0), stop=(c == NCD - 1))
                    for c in range(NCD):
                        mm(p2[:], wq_sb["w2", c][:, ms],
                           xq_ch[:, c * CH:(c + 1) * CH],
                           start=(c == 0), stop=(c == NCD - 1))
                    a1 = t2.tile([P, CH], F32, tag="qa1", name="qa1")
                    nc.scalar.activation(a1[:], p1[:], ACTF.Sigmoid,
                                         bias=bcol["bq1"][:, mc:mc + 1])
                    s1 = t2.tile([P, CH], F32, tag="qs1", name="qs1")
                    nc.vector.scalar_tensor_tensor(s1[:], p1[:],
                                                   bcol["bq1"][:, mc:mc + 1], a1[:],
                                                   ALU.add, ALU.mult)
                    qg = t2.tile([P, CH], F32, tag="qg", name="qg", bufs=NCD)
                    nc.vector.scalar_tensor_tensor(qg[:], p2[:],
                                                   bcol["bq2"][:, mc:mc + 1], s1[:],
                                                   ALU.add, ALU.mult)
                    tmin = t2.tile([P, CH], F32, tag="qtmin", name="qtmin", bufs=NCD)
                    nc.vector.tensor_scalar_min(tmin[:], qg[:], 0.0)
                    qgs.append(qg)
                    qtmins.append(tmin)
                    if pending is not None:
                        tail_pair(p_phi, p_r2s, p_attn, mc)
                for mc in range(NCD):  # Exp batch + phi assembly
                    texp = t2.tile([P, CH], F32, tag="qtexp", name="qtexp")
                    nc.scalar.activation(texp[:], qtmins[mc][:], ACTF.Exp)
                    trel = t2.tile([P, CH], F32, tag="qtrel", name="qtrel")
                    nc.vector.tensor_scalar_max(trel[:], qgs[mc][:], 0.0)
                    nc.vector.tensor_tensor(phi_q[mc][:], texp[:], trel[:], ALU.add)
                if pending is not None:
                    tail_out(p_ch, p_attn)
                pending = (ch, phi_q)
            if has2:
                p_ch, p_phi = pending
                p_r2s, p_attn = tail_head(p_phi)
                for pair in range(NCD):
                    tail_pair(p_phi, p_r2s, p_attn, pair)
                tail_out(p_ch, p_attn)


_CACHE = {}


def _get_nc(single_core=False):
    key = bool(single_core)
    if key not in _CACHE:
        _CACHE[key] = build(single_core)
    return _CACHE[key]


def make_in_maps(inputs):
    f = np.float32
    q = np.asarray(inputs["query"], f).reshape(B * S, D)
    k = np.asarray(inputs["key"], f).reshape(B * S, D)
    v = np.asarray(inputs["value"], f).reshape(B * S, D)
    mask = np.asarray(inputs["mask"], f).reshape(B * S)
    shared = {
        "wq1T": np.ascontiguousarray(np.asarray(inputs["q_w1"], f).T),
        "wq2T": np.ascontiguousarray(np.asarray(inputs["q_w2"], f).T),
        "wk1T": np.ascontiguousarray(np.asarray(inputs["k_w1"], f).T),
        "wk2T": np.ascontiguousarray(np.asarray(inputs["k_w2"], f).T),
        "wv1T": np.ascontiguousarray(np.asarray(inputs["v_w1"], f).T),
        "wv2T": np.ascontiguousarray(np.asarray(inputs["v_w2"], f).T),
        "woT": np.ascontiguousarray(np.asarray(inputs["out_w"], f).T),
        "bq1c": np.ascontiguousarray(np.asarray(inputs["q_b1"], f).reshape(NCD, P).T),
        "bq2c": np.ascontiguousarray(np.asarray(inputs["q_b2"], f).reshape(NCD, P).T),
        "bk1r": np.tile(np.asarray(inputs["k_b1"], f)[None, :], (P, 1)),
        "bk2r": np.tile(np.asarray(inputs["k_b2"], f)[None, :], (P, 1)),
        "bv1r": np.tile(np.asarray(inputs["v_b1"], f)[None, :], (P, 1)),
        "bv2r": np.tile(np.asarray(inputs["v_b2"], f)[None, :], (P, 1)),
        "bor": np.tile(np.asarray(inputs["out_b"], f)[None, :], (P, 1)),
        "ones_row": np.ones((1, P), f),
        "zeros16": np.zeros((P, H), f),
        "ones_col_bf": np.ones((P, 1), ml_dtypes.bfloat16),
    }
    in_maps = []
    for c in range(NCORES):
        sl = slice(c * T, (c + 1) * T)
        m = dict(shared)
        m["xqT"] = np.ascontiguousarray(q[sl].T)
        m["xkT"] = np.ascontiguousarray(k[sl].T)
        m["xvT"] = np.ascontiguousarray(v[sl].T)
        m["maskp"] = np.ascontiguousarray(mask[sl].reshape(NM, P).T)
        in_maps.append(m)
    return in_maps


def kernel(**inputs):
    nc = _get_nc(False)
    in_maps = make_in_maps(inputs)
    res = run_bass_kernel_spmd(nc, in_maps, list(range(NCORES))).results
    outc = np.concatenate([res[c]["out"] for c in range(NCORES)], axis=0)
    return outc.reshape(B, S, D)



# revision 11
# speedup vs baseline: 1.9650x; 1.9650x over previous
"""MultiHeadLinearAttention Trainium2 kernel (8-core SPMD, fp8 DoubleRow).

Sharding: 16384 tokens split across 8 cores (core c: batch c//2, sequence half
c%2). All projections/attention/out-proj are local; the only cross-core
dependency is the per-batch KV summary (kv+ksum packed [256,260]) reduced via a
pair-wise AllReduce.

Numerics (validated in numpy sim; rel err ~1e-2 vs 2e-2 gate):
  - x and all weights pre-quantized host-side to fp8e4m3; weights scaled x64
    (uniform(-1/32,1/32) would be subnormal in fp8 otherwise)
  - all 7 projections run as fp8 DoubleRow matmuls (K=256/instr, 0.5 cyc/row)
  - q,k GLUs single-pass fp8 (their errors cancel in the num/z ratio);
    v GLU and out-proj use weight-split error feedback (W ~ W_hi + W_lo)
  - GLU intermediates bf16 (DVE 2x/4x modes need 2-byte dtypes);
    kv/z/num matmuls bf16; r kept fp32; attn stored as fp8(64*attn)
  - biases enter PSUM via K=1 DoubleRow matmuls (token-major k/v) or ACT
    bias (feature-major q); out bias + 1/4096 unscale applied on host

Engine balance: PE ~170us is the roof; elementwise split DVE/ACT/Pool:
  - ACT: silu/exp only (exp batched by groups of 8 tiles / chunk pairs --
    silu and exp live in different act tables, each switch costs 1283ns)
  - DVE: GLU products (psum reads), phi assembly (bf16 ts/tt fast modes),
    attn eviction, half the out evictions
  - Pool/gpsimd: 1/z partition-broadcast (replaces PE broadcast matmuls),
    weight DMA issue, collective
"""
import os
from contextlib import ExitStack

import ml_dtypes
import numpy as np
import bass_rust
import concourse.bass as bass
import concourse.mybir as mybir
import concourse.tile as tile
from concourse import bacc
from concourse.bass_utils import run_bass_kernel_spmd

F32 = mybir.dt.float32
BF16 = mybir.dt.bfloat16
F8 = mybir.dt.float8e4
ACTF = mybir.ActivationFunctionType
ALU = mybir.AluOpType
DR = mybir.MatmulPerfMode.DoubleRow

B, S, D, H = 4, 4096, 1024, 16
DK = D // H          # 64
EPS = 1e-6
NCORES = 8
T = B * S // NCORES  # 2048 tokens per core
P = 128
NM = T // P          # 16 token tiles
NCD = D // P         # 8 d-chunks
CH = 256             # stage-2 token chunk
NCH = T // CH        # 8 chunks
GB = 8               # stage-1a exp batch group size
GROUPS = [[0, 1], [2, 3], [4, 5], [6, 7]]

V_SPLIT = True       # weight-split error feedback on the v GLU
WO_SPLIT = True      # weight-split error feedback on the out projection

NP8 = ml_dtypes.float8_e4m3


def build(single_core=False):
    nc = bacc.Bacc("TRN2", target_bir_lowering=False, debug=False,
                   num_devices=1 if single_core else NCORES)
    dt_in = {}

    def inp(name, shape, dtype=F8):
        dt_in[name] = nc.dram_tensor(name, shape, dtype, kind="ExternalInput").ap()

    inp("xk8", [NM, P, NCD, P])
    inp("xv8", [NM, P, NCD, P])
    inp("xq8", [NCH, P, NCD, CH])
    for nm in ("wk1", "wk2", "wq1", "wq2"):
        inp(nm, [P, NCD, D])
    vws = ["wv1h", "wv2h"] + (["wv1l", "wv2l"] if V_SPLIT else [])
    for nm in vws:
        inp(nm, [P, NCD, D])
    ows = ["woh"] + (["wol"] if WO_SPLIT else [])
    for nm in ows:
        inp(nm, [P, NCD, D])
    for nm in ("bk1", "bk2", "bv1", "bv2"):
        inp(nm, [1, 2, D])
    inp("ones2", [1, 2, P])
    inp("bq1c", [P, NCD], F32)
    inp("bq2c64", [P, NCD], F32)
    inp("maskp", [P, NM], F32)
    out = nc.dram_tensor("out", [T, D], F32, kind="ExternalOutput").ap()

    with tile.TileContext(nc) as tc:
        _emit(nc, tc, dt_in, out, single_core)
    nc.compile()
    return nc


def _emit(nc, tc, dt, out, single_core):
    def mm(ps, l, r, start, stop, skip=False):
        nc.tensor.matmul(ps, l, r, start=start, stop=stop, perf_mode=DR,
                         skip_group_check=skip)

    # chain ACT instructions so the scheduler preserves emission order --
    # silu and exp live in different act-func tables (1283ns per switch), and
    # the list scheduler otherwise interleaves them freely
    last_act = [None]

    def act(out_ap, in_ap, func, **kw):
        inst = nc.scalar.activation(out_ap, in_ap, func, **kw)
        if last_act[0] is not None:
            bass_rust.add_dep_helper(inst.ins, last_act[0].ins,
                                     reason="act-table-order")
        last_act[0] = inst
        return inst

    with ExitStack() as st0:
        const = st0.enter_context(tc.tile_pool(name="const", bufs=1))
        dram = st0.enter_context(tc.tile_pool(name="dram", bufs=1, space="DRAM"))
        kvres = st0.enter_context(tc.tile_pool(name="kvres", bufs=1))

        ones2_sb = const.tile([1, 2, P], F8, name="ones2_sb")
        nc.scalar.dma_start(ones2_sb[:], dt["ones2"][:])
        bq1c_sb = const.tile([P, NCD], F32, name="bq1c_sb")
        nc.scalar.dma_start(bq1c_sb[:], dt["bq1c"][:])
        bq2c64_sb = const.tile([P, NCD], F32, name="bq2c64_sb")
        nc.scalar.dma_start(bq2c64_sb[:], dt["bq2c64"][:])
        maskp_sb = const.tile([P, NM], F32, name="maskp_sb")
        nc.scalar.dma_start(maskp_sb[:], dt["maskp"][:])
        ones16 = const.tile([P, H], BF16, name="ones16")
        nc.gpsimd.memset(ones16[:], 1.0)

        # stage-2 weight pool at st0 scope so DMAs can prefetch during stage 1b
        wqop = st0.enter_context(tc.tile_pool(name="wqop", bufs=1, side="right"))

        kvstage_ctx = ExitStack()
        kvstage = kvstage_ctx.enter_context(tc.tile_pool(name="kvstage", bufs=1))

        st1 = st0.enter_context(ExitStack())
        phik_pool = st1.enter_context(tc.tile_pool(name="phik", bufs=1))
        phi_k = [phik_pool.tile([P, D], BF16, tag=f"phik_{m}", name=f"phik_{m}")
                 for m in range(NM)]

        # wv pool spans stage 1a (prefetch) + stage 1b (use)
        stv = ExitStack()
        wvp = stv.enter_context(tc.tile_pool(name="wv", bufs=1))

        # ================= stage 1a: k projection -> phi_k =================
        with ExitStack() as st1a:
            wkp = st1a.enter_context(tc.tile_pool(name="wk", bufs=1))
            xkp = st1a.enter_context(tc.tile_pool(name="xk", bufs=2))
            g1p = st1a.enter_context(tc.tile_pool(name="g1p", bufs=2))
            kgp = st1a.enter_context(tc.tile_pool(name="kgp", bufs=2))
            mintp = st1a.enter_context(tc.tile_pool(name="mintp", bufs=GB + 2))
            trelp = st1a.enter_context(tc.tile_pool(name="trelp", bufs=GB + 2))
            texpp = st1a.enter_context(tc.tile_pool(name="texpp", bufs=2))
            pk1 = st1a.enter_context(tc.tile_pool(name="pk1", bufs=4, space="PSUM"))
            pk2 = st1a.enter_context(tc.tile_pool(name="pk2", bufs=4, space="PSUM"))

            wk_sb = {}
            for w, src in ((0, "wk1"), (1, "wk2")):
                wk_sb[w] = wkp.tile([P, NCD, D], F8, tag=f"wk{w}", name=f"wk{w}")
                nc.sync.dma_start(wk_sb[w][:], dt[src][:])
            bk_sb = {}
            for w, src in ((0, "bk1"), (1, "bk2")):
                bk_sb[w] = wkp.tile([1, 2, D], F8, tag=f"bk{w}", name=f"bk{w}")
                nc.sync.dma_start(bk_sb[w][:], dt[src][:])
            wv_sb = {}
            vnames = [("1h", "wv1h"), ("2h", "wv2h")]
            if V_SPLIT:
                vnames += [("1l", "wv1l"), ("2l", "wv2l")]
            for w, src in vnames:
                wv_sb[w] = wvp.tile([P, NCD, D], F8, tag=f"wv{w}", name=f"wv{w}")
                nc.sync.dma_start(wv_sb[w][:], dt[src][:])
            bv_sb = {}
            for w, src in ((0, "bv1"), (1, "bv2")):
                bv_sb[w] = wvp.tile([1, 2, D], F8, tag=f"bv{w}", name=f"bv{w}")
                nc.sync.dma_start(bv_sb[w][:], dt[src][:])

            mints, trels = {}, {}

            def phi_flush(ms):
                for m2 in ms:
                    texp = texpp.tile([P, D], BF16, tag="texp", name="texp")
                    act(texp[:], mints[m2][:], ACTF.Exp, scale=1.0 / 64)
                    nc.vector.tensor_tensor(phi_k[m2][:], texp[:], trels[m2][:],
                                            ALU.add)

            for m in range(NM):
                xk_m = xkp.tile([P, NCD, P], F8, tag="xk", name="xk")
                nc.sync.dma_start(xk_m[:], dt["xk8"][m, :, :, :])
                kg = kgp.tile([P, D], BF16, tag="kg", name="kg")
                g1 = g1p.tile([P, D], BF16, tag="g1", name="g1")
                for half in range(2):
                    ns = slice(half * 512, (half + 1) * 512)
                    p1 = pk1.tile([P, 512], F32, tag="p1", name="p1")
                    p2 = pk2.tile([P, 512], F32, tag="p2", name="p2")
                    mm(p1[:], ones2_sb[:], bk_sb[0][:, :, ns], True, False)
                    for cp in range(4):
                        mm(p1[:], xk_m[:, 2 * cp:2 * cp + 2, :],
                           wk_sb[0][:, 2 * cp:2 * cp + 2, ns], False, cp == 3)
                    mm(p2[:], ones2_sb[:], bk_sb[1][:, :, ns], True, False)
                    for cp in range(4):
                        mm(p2[:], xk_m[:, 2 * cp:2 * cp + 2, :],
                           wk_sb[1][:, 2 * cp:2 * cp + 2, ns], False, cp == 3)
                    act(g1[:, ns], p1[:], ACTF.Silu, scale=1.0 / 64)
                    nc.vector.tensor_tensor(kg[:, ns], g1[:, ns], p2[:], ALU.mult)
                mint = mintp.tile([P, D], BF16, tag="mint", name="mint")
                nc.vector.tensor_scalar_min(mint[:], kg[:], 0.0)
                trel = trelp.tile([P, D], BF16, tag="trel", name="trel")
                nc.vector.tensor_scalar(trel[:], kg[:], 0.0, 1.0 / 64,
                                        ALU.max, ALU.mult)
                mints[m], trels[m] = mint, trel
                if m % GB == GB - 1:
                    phi_flush(range(m - GB + 1, m + 1))

        # prefetch stage-2 weights during stage 1b
        wq_sb = {}
        for w, src in ((0, "wq1"), (1, "wq2")):
            wq_sb[w] = wqop.tile([P, NCD, D], F8, tag=f"wq{w}", name=f"wq{w}")
            nc.sync.dma_start(wq_sb[w][:], dt[src][:])
        wo_sb = []
        for src in (["woh"] + (["wol"] if WO_SPLIT else [])):
            t = wqop.tile([P, NCD, D], F8, tag=src, name=src)
            nc.sync.dma_start(t[:], dt[src][:])
            wo_sb.append(t)

        # ============== stage 1b: v projection + kv/ksum accumulation ========
        with ExitStack() as st1b:
            xvp = st1b.enter_context(tc.tile_pool(name="xv", bufs=2))
            g1vp = st1b.enter_context(tc.tile_pool(name="g1v", bufs=3))
            vgp = st1b.enter_context(tc.tile_pool(name="vgp", bufs=1))
            pv1 = st1b.enter_context(tc.tile_pool(name="pv1", bufs=3, space="PSUM"))
            pv2 = st1b.enter_context(tc.tile_pool(name="pv2", bufs=3, space="PSUM"))
            pkvp = st1b.enter_context(tc.tile_pool(name="pkv", bufs=1, space="PSUM"))

            vg_bufs = [vgp.tile([P, H, 65], BF16, tag=f"vg{i}", name=f"vg{i}")
                       for i in range(3)]
            psum_kv = [pkvp.tile([P, 260], F32, tag=f"pkv{i}", name=f"pkv{i}")
                       for i in range(2)]

            def kv_tail(m):
                vg = vg_bufs[m % 3]
                for h in range(H):
                    bank = psum_kv[h // 8]
                    pr = (h % 2) * 64
                    fc = ((h // 2) % 4) * 65
                    # HW start=True marks the WHOLE psum row (all columns) of
                    # the participating partitions pending-zero -- start only
                    # on the first head per (bank, partition-half); later
                    # heads' first writes overwrite via has_written
                    first = m == 0 and (h % 8) < 2
                    last = m == NM - 1 and (h % 8) >= 6
                    nc.tensor.matmul(
                        bank[pr:pr + 64, fc:fc + 65],
                        phi_k[m][:, h * DK:(h + 1) * DK],
                        vg[:, h:h + 1, :],
                        start=first, stop=last,
                        skip_group_check=not (first or last))

            for m in range(NM):
                xv_m = xvp.tile([P, NCD, P], F8, tag="xv", name="xv")
                nc.sync.dma_start(xv_m[:], dt["xv8"][m, :, :, :])
                vg = vg_bufs[m % 3]
                # ksum column: phi_k column of ones * mask (handles masking)
                nc.vector.tensor_scalar_mul(vg[:, :, 64:65], ones16[:],
                                            maskp_sb[:, m:m + 1])
                for half in range(2):
                    ns = slice(half * 512, (half + 1) * 512)
                    p1 = pv1.tile([P, 512], F32, tag="pv1", name="pv1")
                    p2 = pv2.tile([P, 512], F32, tag="pv2", name="pv2")
                    mm(p1[:], ones2_sb[:], bv_sb[0][:, :, ns], True, False)
                    for cp in range(4):
                        mm(p1[:], xv_m[:, 2 * cp:2 * cp + 2, :],
                           wv_sb["1h"][:, 2 * cp:2 * cp + 2, ns], False,
                           (not V_SPLIT) and cp == 3)
                    if V_SPLIT:
                        for cp in range(4):
                            mm(p1[:], xv_m[:, 2 * cp:2 * cp + 2, :],
                               wv_sb["1l"][:, 2 * cp:2 * cp + 2, ns], False,
                               cp == 3)
                    mm(p2[:], ones2_sb[:], bv_sb[1][:, :, ns], True, False)
                    for cp in range(4):
                        mm(p2[:], xv_m[:, 2 * cp:2 * cp + 2, :],
                           wv_sb["2h"][:, 2 * cp:2 * cp + 2, ns], False,
                           (not V_SPLIT) and cp == 3)
                    if V_SPLIT:
                        for cp in range(4):
                            mm(p2[:], xv_m[:, 2 * cp:2 * cp + 2, :],
                               wv_sb["2l"][:, 2 * cp:2 * cp + 2, ns], False,
                               cp == 3)
                    g1v = g1vp.tile([P, 512], BF16, tag="g1v", name="g1v")
                    act(g1v[:], p1[:], ACTF.Silu, scale=1.0 / 64)
                    # vg = (silu * mask) * p2  (64-scaled; mask folded here)
                    nc.vector.scalar_tensor_tensor(
                        vg[:, 8 * half:8 * half + 8, 0:64], g1v[:],
                        maskp_sb[:, m:m + 1], p2[:], ALU.mult, ALU.mult)
                if m >= 2:
                    kv_tail(m - 2)
            kv_tail(NM - 2)
            kv_tail(NM - 1)
            kvev = [kvstage.tile([P, 260], F32, tag=f"kvev{i}", name=f"kvev{i}")
                    for i in range(2)]
            for i in range(2):
                nc.vector.tensor_copy(kvev[i][:], psum_kv[i][:])

        stv.close()  # frees wv weights
        st1.close()  # frees phi_k SBUF before stage 2

        # ============ collective: pair AllReduce of kv+ksum ============
        cc_in = dram.tile([2 * P, 260], F32)
        cc_out = dram.tile([2 * P, 260], F32)
        nc.gpsimd.dma_start(cc_in[0:P, :], kvev[0][:])
        nc.gpsimd.dma_start(cc_in[P:2 * P, :], kvev[1][:])
        kvstage_ctx.close()
        if single_core:
            nc.gpsimd.dma_start(cc_out[:], cc_in[:])
        else:
            nc.gpsimd.collective_compute(
                "AllReduce", ALU.add, replica_groups=GROUPS,
                ins=[cc_in.opt()], outs=[cc_out.opt()])

        # repack: kv -> block-diag bf16 tiles; ksum -> block-diag bf16 tiles
        kv_bd = [kvres.tile([P, P], BF16, tag=f"kvbd{c}", name=f"kvbd{c}")
                 for c in range(NCD)]
        ksum_bd = [kvres.tile([P, H], BF16, tag=f"ksbd{c}", name=f"ksbd{c}")
                   for c in range(NCD)]
        with ExitStack() as strp:
            rpp = strp.enter_context(tc.tile_pool(name="rpp", bufs=1))
            kvf32 = rpp.tile([P, NCD, DK], F32, name="kvf32")
            ksf32 = rpp.tile([P, NCD], F32, name="ksf32")
            # cc_out(row=p [+128], col=cp*65+j) is affine in (p, cp, j):
            # head h=2cp+p//64 lives at row (h//8)*128+(h%2)*64+(p%64) = p [+128]
            base = cc_out
            for lo in range(2):
                off = lo * 4 * 65 * 0 + lo * P * 260  # high half: rows 128..255
                cps = slice(4 * lo, 4 * lo + 4)
                src_kv = bass.AP(base.tensor, base.offset + off,
                                 [[260, P], [65, 4], [1, DK]])
                nc.sync.dma_start(kvf32[:, cps, :], src_kv)
                src_ks = bass.AP(base.tensor, base.offset + off + DK,
                                 [[260, P], [65, 4], [1, 1]])
                nc.sync.dma_start(ksf32[:, cps], src_ks)
            for cp in range(NCD):
                nc.gpsimd.memset(kv_bd[cp][:], 0.0)
                nc.vector.tensor_copy(kv_bd[cp][0:64, 0:64],
                                      kvf32[0:64, cp:cp + 1, :])
                nc.vector.tensor_copy(kv_bd[cp][64:128, 64:128],
                                      kvf32[64:128, cp:cp + 1, :])
                nc.gpsimd.memset(ksum_bd[cp][:], 0.0)
                nc.vector.tensor_copy(ksum_bd[cp][0:64, 2 * cp:2 * cp + 1],
                                      ksf32[0:64, cp:cp + 1])
                nc.vector.tensor_copy(ksum_bd[cp][64:128, 2 * cp + 1:2 * cp + 2],
                                      ksf32[64:128, cp:cp + 1])

            # ============ stage 2: q -> phi_q -> z -> attn -> out ============
            with ExitStack() as st2:
                xqp = st2.enter_context(tc.tile_pool(name="xq", bufs=2))
                g1qp = st2.enter_context(tc.tile_pool(name="g1q", bufs=2))
                qgp = st2.enter_context(tc.tile_pool(name="qg", bufs=4))
                mint2 = st2.enter_context(tc.tile_pool(name="mint2", bufs=2))
                texp2 = st2.enter_context(tc.tile_pool(name="texp2", bufs=2))
                trel2 = st2.enter_context(tc.tile_pool(name="trel2", bufs=2))
                phiqp = st2.enter_context(tc.tile_pool(name="phiq", bufs=4))
                zepsp = st2.enter_context(tc.tile_pool(name="zeps", bufs=2))
                rsbp = st2.enter_context(tc.tile_pool(name="rsb", bufs=2))
                rrepp = st2.enter_context(tc.tile_pool(name="rrep", bufs=2))
                rdram = st2.enter_context(tc.tile_pool(name="rdram", bufs=2,
                                                       space="DRAM"))
                attnp = st2.enter_context(tc.tile_pool(name="attn", bufs=2))
                osbp = st2.enter_context(tc.tile_pool(name="osb", bufs=2))
                pq1 = st2.enter_context(tc.tile_pool(name="pq1", bufs=2,
                                                     space="PSUM"))
                pq2 = st2.enter_context(tc.tile_pool(name="pq2", bufs=2,
                                                     space="PSUM"))
                pzp = st2.enter_context(tc.tile_pool(name="pz", bufs=1,
                                                     space="PSUM"))
                pnp = st2.enter_context(tc.tile_pool(name="pn", bufs=2,
                                                     space="PSUM"))
                pop = st2.enter_context(tc.tile_pool(name="po", bufs=1,
                                                     space="PSUM"))

                def proj(ch):
                    xq_ch = xqp.tile([P, NCD, CH], F8, tag="xq", name="xq")
                    nc.sync.dma_start(xq_ch[:], dt["xq8"][ch, :, :, :])
                    qg = qgp.tile([P, NCD, CH], BF16, tag="qg", name="qg")
                    for mc in range(NCD):
                        ms = slice(mc * P, (mc + 1) * P)
                        p1 = pq1.tile([P, CH], F32, tag="pq1", name="pq1")
                        p2 = pq2.tile([P, CH], F32, tag="pq2", name="pq2")
                        for cp in range(4):
                            mm(p1[:], wq_sb[0][:, 2 * cp:2 * cp + 2, ms],
                               xq_ch[:, 2 * cp:2 * cp + 2, :], cp == 0, cp == 3)
                        for cp in range(4):
                            mm(p2[:], wq_sb[1][:, 2 * cp:2 * cp + 2, ms],
                               xq_ch[:, 2 * cp:2 * cp + 2, :], cp == 0, cp == 3)
                        g1 = g1qp.tile([P, CH], BF16, tag="g1q", name="g1q")
                        act(g1[:], p1[:], ACTF.Silu,
                            bias=bq1c_sb[:, mc:mc + 1], scale=1.0 / 64)
                        nc.vector.scalar_tensor_tensor(
                            qg[:, mc:mc + 1, :], p2[:],
                            bq2c64_sb[:, mc:mc + 1], g1[:], ALU.add, ALU.mult)
                    return qg

                def phiq_build(qg):
                    mint = mint2.tile([P, NCD, CH], BF16, tag="mintq",
                                      name="mintq")
                    nc.vector.tensor_scalar_min(mint[:], qg[:], 0.0)
                    texp = texp2.tile([P, NCD, CH], BF16, tag="texpq",
                                      name="texpq")
                    act(texp[:], mint[:], ACTF.Exp, scale=1.0 / 64)
                    trel = trel2.tile([P, NCD, CH], BF16, tag="trelq",
                                      name="trelq")
                    nc.vector.tensor_scalar(trel[:], qg[:], 0.0, 1.0 / 64,
                                            ALU.max, ALU.mult)
                    phiq = phiqp.tile([P, NCD, CH], BF16, tag="phiq",
                                      name="phiq")
                    nc.vector.tensor_tensor(phiq[:], texp[:], trel[:], ALU.add)
                    return phiq

                def rhead(ch, phiq):
                    pz = pzp.tile([H, CH], F32, tag="pz", name="pz")
                    for cp in range(NCD):
                        nc.tensor.matmul(pz[:], ksum_bd[cp][:],
                                         phiq[:, cp:cp + 1, :],
                                         start=cp == 0, stop=cp == NCD - 1)
                    zeps = zepsp.tile([H, CH], F32, tag="zeps", name="zeps")
                    nc.vector.tensor_scalar_add(zeps[:], pz[:], EPS)
                    rsb = rsbp.tile([H, CH], F32, tag="rsb", name="rsb")
                    nc.vector.reciprocal(rsb[:], zeps[:])
                    # broadcast r across partitions via a DRAM round-trip: a
                    # stride-0 partition AP replicates row 2cp(+1) to 64 rows;
                    # launched a full chunk-pair before its consumers so the
                    # ~4us round trip never blocks the PE
                    rd = rdram.tile([H, CH], F32, tag="rd", name="rd")
                    nc.sync.dma_start(rd[:], rsb[:])
                    rrep = rrepp.tile([P, NCD, CH], F32, tag="rrep", name="rrep")
                    base = rd[:]
                    src_lo = bass.AP(base.tensor, base.offset,
                                     [[0, 64], [2 * CH, NCD], [1, CH]])
                    src_hi = bass.AP(base.tensor, base.offset + CH,
                                     [[0, 64], [2 * CH, NCD], [1, CH]])
                    nc.sync.dma_start(rrep[0:64, :, :], src_lo)
                    nc.sync.dma_start(rrep[64:128, :, :], src_hi)
                    return rrep

                def tail2(ch, phiq, rrep):
                    attn = attnp.tile([P, NCD, CH], F8, tag="attn", name="attn")
                    for cp in range(NCD):
                        pn = pnp.tile([P, CH], F32, tag="pn", name="pn")
                        nc.tensor.matmul(pn[:], kv_bd[cp][:],
                                         phiq[:, cp:cp + 1, :],
                                         start=True, stop=True)
                        nc.vector.tensor_tensor(attn[:, cp:cp + 1, :], pn[:],
                                                rrep[:, cp:cp + 1, :], ALU.mult)
                    for mt in range(2):
                        for nh in range(2):
                            ns = slice(nh * 512, (nh + 1) * 512)
                            po = pop.tile([P, 512], F32, tag="po", name="po")
                            nmm = len(wo_sb) * 4
                            i = 0
                            for wo_t in wo_sb:
                                for cp in range(4):
                                    mm(po[:],
                                       attn[:, 2 * cp:2 * cp + 2,
                                            mt * P:(mt + 1) * P],
                                       wo_t[:, 2 * cp:2 * cp + 2, ns],
                                       i == 0, i == nmm - 1)
                                    i += 1
                            osb = osbp.tile([P, 512], F32, tag="osb", name="osb")
                            if (mt + nh) % 2 == 0:
                                nc.scalar.activation(osb[:], po[:], ACTF.Copy)
                            else:
                                nc.vector.tensor_copy(osb[:], po[:])
                            row0 = ch * CH + mt * P
                            nc.gpsimd.dma_start(out[row0:row0 + P, ns], osb[:])

                pending = None
                for pr2 in range(NCH // 2):
                    qgs = [proj(2 * pr2), proj(2 * pr2 + 1)]
                    if pending is not None:
                        for c2, ph2, rr2 in pending:
                            tail2(c2, ph2, rr2)
                    phis = [phiq_build(qgs[i]) for i in range(2)]
                    pending = [(2 * pr2 + i, phis[i],
                                rhead(2 * pr2 + i, phis[i])) for i in range(2)]
                for c2, ph2, rr2 in pending:
                    tail2(c2, ph2, rr2)


# revision 20
# speedup vs baseline: 2.3631x; 1.2026x over previous
"""MultiHeadLinearAttention Trainium2 kernel (8-core SPMD, fp8 DoubleRow).

Sharding: 16384 tokens split across 8 cores (core c: batch c//2, sequence half
c%2). All projections/attention/out-proj are local; the only cross-core
dependency is the per-batch KV summary (kv+ksum packed [256,260]) reduced via a
pair-wise AllReduce.

Numerics (validated in numpy sim; rel err ~1e-2 vs 2e-2 gate):
  - x and all weights pre-quantized host-side to fp8e4m3; weights scaled x64
    (uniform(-1/32,1/32) would be subnormal in fp8 otherwise)
  - all 7 projections run as fp8 DoubleRow matmuls (K=256/instr, 0.5 cyc/row)
  - q,k GLUs single-pass fp8 (their errors cancel in the num/z ratio);
    v GLU and out-proj use weight-split error feedback (W ~ W_hi + W_lo)
  - GLU intermediates bf16 (DVE 2x/4x modes need 2-byte dtypes);
    kv/z/num matmuls bf16; r kept fp32; attn stored as fp8(64*attn)
  - biases enter PSUM via K=1 DoubleRow matmuls (token-major k/v) or ACT
    bias (feature-major q); out bias + 1/4096 unscale applied on host

Engine balance: PE ~170us is the roof; elementwise split DVE/ACT/Pool:
  - ACT: silu/exp only (exp batched by groups of 8 tiles / chunk pairs --
    silu and exp live in different act tables, each switch costs 1283ns)
  - DVE: GLU products (psum reads), phi assembly (bf16 ts/tt fast modes),
    attn eviction, half the out evictions
  - Pool/gpsimd: 1/z partition-broadcast (replaces PE broadcast matmuls),
    weight DMA issue, collective
"""
import os
from contextlib import ExitStack

import ml_dtypes
import numpy as np
import bass_rust
import concourse.bass as bass
import concourse.mybir as mybir
import concourse.tile as tile
from concourse import bacc
from concourse.bass_utils import run_bass_kernel_spmd

F32 = mybir.dt.float32
BF16 = mybir.dt.bfloat16
F8 = mybir.dt.float8e4
ACTF = mybir.ActivationFunctionType
ALU = mybir.AluOpType
DR = mybir.MatmulPerfMode.DoubleRow

B, S, D, H = 4, 4096, 1024, 16
DK = D // H          # 64
EPS = 1e-6
NCORES = 8
T = B * S // NCORES  # 2048 tokens per core
P = 128
NM = T // P          # 16 token tiles
NCD = D // P         # 8 d-chunks
CH = 256             # stage-2 token chunk
NCH = T // CH        # 8 chunks
GB = 8               # stage-1a exp batch group size
GROUPS = [[0, 1], [2, 3], [4, 5], [6, 7]]

V_SPLIT = False      # single-pass fp8 v GLU (stage-1 is PE-bound; error ~11.4e-3)
WO_SPLIT = True      # weight-split error feedback on the out projection

NP8 = ml_dtypes.float8_e4m3


def build(single_core=False):
    nc = bacc.Bacc("TRN2", target_bir_lowering=False, debug=False,
                   num_devices=1 if single_core else NCORES)
    dt_in = {}

    def inp(name, shape, dtype=F8):
        dt_in[name] = nc.dram_tensor(name, shape, dtype, kind="ExternalInput").ap()

    inp("xk8", [NM, P, NCD, P])
    inp("xv8", [NM, P, NCD, P])
    inp("xq8", [NCH, P, NCD, CH])
    for nm in ("wk1", "wk2", "wq1", "wq2"):
        inp(nm, [P, NCD, D])
    vws = ["wv1h", "wv2h"] + (["wv1l", "wv2l"] if V_SPLIT else [])
    for nm in vws:
        inp(nm, [P, NCD, D])
    ows = ["woh"] + (["wol"] if WO_SPLIT else [])
    for nm in ows:
        inp(nm, [P, NCD, D])
    for nm in ("bk1", "bk2", "bv1", "bv2"):
        inp(nm, [1, 2, D])
    inp("ones2", [1, 2, P])
    inp("bq1c", [P, NCD], F32)
    inp("bq2c64", [P, NCD], F32)
    inp("maskp", [P, NM], F32)
    out = nc.dram_tensor("out", [T, D], F32, kind="ExternalOutput").ap()

    with tile.TileContext(nc) as tc:
        _emit(nc, tc, dt_in, out, single_core)
    nc.compile()
    return nc


def _emit(nc, tc, dt, out, single_core):
    def mm(ps, l, r, start, stop, skip=False):
        nc.tensor.matmul(ps, l, r, start=start, stop=stop, perf_mode=DR,
                         skip_group_check=skip)

    # chain ACT instructions so the scheduler preserves emission order --
    # silu and exp live in different act-func tables (1283ns per switch), and
    # the list scheduler otherwise interleaves them freely
    last_act = [None]

    def act(out_ap, in_ap, func, **kw):
        inst = nc.scalar.activation(out_ap, in_ap, func, **kw)
        if last_act[0] is not None:
            bass_rust.add_dep_helper(inst.ins, last_act[0].ins,
                                     reason="act-table-order")
        last_act[0] = inst
        return inst

    with ExitStack() as st0:
        const = st0.enter_context(tc.tile_pool(name="const", bufs=1))
        dram = st0.enter_context(tc.tile_pool(name="dram", bufs=1, space="DRAM"))
        kvres = st0.enter_context(tc.tile_pool(name="kvres", bufs=1))

        ones2_sb = const.tile([1, 2, P], F8, name="ones2_sb")
        nc.scalar.dma_start(ones2_sb[:], dt["ones2"][:])
        bq1c_sb = const.tile([P, NCD], F32, name="bq1c_sb")
        nc.scalar.dma_start(bq1c_sb[:], dt["bq1c"][:])
        bq2c64_sb = const.tile([P, NCD], F32, name="bq2c64_sb")
        nc.scalar.dma_start(bq2c64_sb[:], dt["bq2c64"][:])
        maskp_sb = const.tile([P, NM], F32, name="maskp_sb")
        nc.scalar.dma_start(maskp_sb[:], dt["maskp"][:])
        ones16 = const.tile([P, H], BF16, name="ones16")
        nc.gpsimd.memset(ones16[:], 1.0)

        # stage-2 weight pool at st0 scope so DMAs can prefetch during stage 1b
        wqop = st0.enter_context(tc.tile_pool(name="wqop", bufs=1, side="right"))

        kvstage = st0.enter_context(tc.tile_pool(name="kvstage", bufs=1))

        st1 = st0.enter_context(ExitStack())
        phik_pool = st1.enter_context(tc.tile_pool(name="phik", bufs=1))
        phi_k = [phik_pool.tile([P, D], BF16, tag=f"phik_{m}", name=f"phik_{m}")
                 for m in range(NM)]

        # wv pool spans stage 1a (prefetch) + stage 1b (use)
        stv = ExitStack()
        wvp = stv.enter_context(tc.tile_pool(name="wv", bufs=1))

        # ================= stage 1a: k projection -> phi_k =================
        with ExitStack() as st1a:
            wkp = st1a.enter_context(tc.tile_pool(name="wk", bufs=1))
            xkp = st1a.enter_context(tc.tile_pool(name="xk", bufs=2))
            g1p = st1a.enter_context(tc.tile_pool(name="g1p", bufs=2))
            kgp = st1a.enter_context(tc.tile_pool(name="kgp", bufs=2))
            mintp = st1a.enter_context(tc.tile_pool(name="mintp", bufs=GB + 2))
            trelp = st1a.enter_context(tc.tile_pool(name="trelp", bufs=GB + 2))
            texpp = st1a.enter_context(tc.tile_pool(name="texpp", bufs=2))
            pk1 = st1a.enter_context(tc.tile_pool(name="pk1", bufs=4, space="PSUM"))
            pk2 = st1a.enter_context(tc.tile_pool(name="pk2", bufs=4, space="PSUM"))

            bk_sb = {}
            for w, src in ((0, "bk1"), (1, "bk2")):
                bk_sb[w] = wkp.tile([1, 2, D], F8, tag=f"bk{w}", name=f"bk{w}")
                nc.sync.dma_start(bk_sb[w][:], dt[src][:])
            xk0 = xkp.tile([P, NCD, P], F8, tag="xk", name="xk0")
            nc.sync.dma_start(xk0[:], dt["xk8"][0, :, :, :])
            wk_sb = {}
            for w, src in ((0, "wk1"), (1, "wk2")):
                wk_sb[w] = wkp.tile([P, NCD, D], F8, tag=f"wk{w}", name=f"wk{w}")
                for hf in range(2):
                    ns = slice(hf * 512, (hf + 1) * 512)
                    nc.sync.dma_start(wk_sb[w][:, :, ns], dt[src][:, :, ns])
            wv_sb = {}
            bv_sb = {}

            def wv_prefetch():
                vnames = [("1h", "wv1h"), ("2h", "wv2h")]
                if V_SPLIT:
                    vnames += [("1l", "wv1l"), ("2l", "wv2l")]
                for w, src in vnames:
                    wv_sb[w] = wvp.tile([P, NCD, D], F8, tag=f"wv{w}",
                                        name=f"wv{w}")
                    nc.scalar.dma_start(wv_sb[w][:], dt[src][:])
                for w, src in ((0, "bv1"), (1, "bv2")):
                    bv_sb[w] = wvp.tile([1, 2, D], F8, tag=f"bv{w}",
                                        name=f"bv{w}")
                    nc.scalar.dma_start(bv_sb[w][:], dt[src][:])

            mints, trels = {}, {}

            def phi_flush(ms):
                for m2 in ms:
                    texp = texpp.tile([P, D], BF16, tag="texp", name="texp")
                    act(texp[:], mints[m2][:], ACTF.Exp, scale=1.0 / 64)
                    nc.vector.tensor_tensor(phi_k[m2][:], texp[:], trels[m2][:],
                                            ALU.add)

            for m in range(NM):
                xk_m = xkp.tile([P, NCD, P], F8, tag="xk", name="xk")
                nc.sync.dma_start(xk_m[:], dt["xk8"][m, :, :, :])
                kg = kgp.tile([P, D], BF16, tag="kg", name="kg")
                g1 = g1p.tile([P, D], BF16, tag="g1", name="g1")
                for half in range(2):
                    ns = slice(half * 512, (half + 1) * 512)
                    p1 = pk1.tile([P, 512], F32, tag="p1", name="p1")
                    p2 = pk2.tile([P, 512], F32, tag="p2", name="p2")
                    mm(p1[:], ones2_sb[:], bk_sb[0][:, :, ns], True, False)
                    for cp in range(4):
                        mm(p1[:], xk_m[:, 2 * cp:2 * cp + 2, :],
                           wk_sb[0][:, 2 * cp:2 * cp + 2, ns], False, cp == 3)
                    mm(p2[:], ones2_sb[:], bk_sb[1][:, :, ns], True, False)
                    for cp in range(4):
                        mm(p2[:], xk_m[:, 2 * cp:2 * cp + 2, :],
                           wk_sb[1][:, 2 * cp:2 * cp + 2, ns], False, cp == 3)
                    act(g1[:, ns], p1[:], ACTF.Silu, scale=1.0 / 64)
                    nc.vector.tensor_tensor(kg[:, ns], g1[:, ns], p2[:], ALU.mult)
                mint = mintp.tile([P, D], BF16, tag="mint", name="mint")
                nc.vector.tensor_scalar_min(mint[:], kg[:], 0.0)
                trel = trelp.tile([P, D], BF16, tag="trel", name="trel")
                nc.vector.tensor_scalar(trel[:], kg[:], 0.0, 1.0 / 64,
                                        ALU.max, ALU.mult)
                mints[m], trels[m] = mint, trel
                if m == 1:
                    wv_prefetch()
                if m % GB == GB - 1:
                    phi_flush(range(m - GB + 1, m + 1))

        # prefetch stage-2 weights during stage 1b
        wq_sb = {}
        for w, src in ((0, "wq1"), (1, "wq2")):
            wq_sb[w] = wqop.tile([P, NCD, D], F8, tag=f"wq{w}", name=f"wq{w}")
            nc.sync.dma_start(wq_sb[w][:], dt[src][:])
        wo_sb = []
        for src in (["woh"] + (["wol"] if WO_SPLIT else [])):
            t = wqop.tile([P, NCD, D], F8, tag=src, name=src)
            nc.sync.dma_start(t[:], dt[src][:])
            wo_sb.append(t)

        # ============== stage 1b: v projection + kv/ksum accumulation ========
        with ExitStack() as st1b:
            xvp = st1b.enter_context(tc.tile_pool(name="xv", bufs=2))
            g1vp = st1b.enter_context(tc.tile_pool(name="g1v", bufs=3))
            vgp = st1b.enter_context(tc.tile_pool(name="vgp", bufs=1))
            pv1 = st1b.enter_context(tc.tile_pool(name="pv1", bufs=3, space="PSUM"))
            pv2 = st1b.enter_context(tc.tile_pool(name="pv2", bufs=3, space="PSUM"))
            pkvp = st1b.enter_context(tc.tile_pool(name="pkv", bufs=1, space="PSUM"))

            vg_bufs = [vgp.tile([P, H, 65], BF16, tag=f"vg{i}", name=f"vg{i}")
                       for i in range(3)]
            psum_kv = [pkvp.tile([P, 260], F32, tag=f"pkv{i}", name=f"pkv{i}")
                       for i in range(2)]

            def kv_tail(m):
                vg = vg_bufs[m % 3]
                for h in range(H):
                    bank = psum_kv[h // 8]
                    pr = (h % 2) * 64
                    fc = ((h // 2) % 4) * 65
                    # HW start=True marks the WHOLE psum row (all columns) of
                    # the participating partitions pending-zero -- start only
                    # on the first head per (bank, partition-half); later
                    # heads' first writes overwrite via has_written
                    first = m == 0 and (h % 8) < 2
                    last = m == NM - 1 and (h % 8) >= 6
                    nc.tensor.matmul(
                        bank[pr:pr + 64, fc:fc + 65],
                        phi_k[m][:, h * DK:(h + 1) * DK],
                        vg[:, h:h + 1, :],
                        start=first, stop=last,
                        skip_group_check=not (first or last))

            for m in range(NM):
                xv_m = xvp.tile([P, NCD, P], F8, tag="xv", name="xv")
                nc.sync.dma_start(xv_m[:], dt["xv8"][m, :, :, :])
                vg = vg_bufs[m % 3]
                # ksum column: phi_k column of ones * mask (handles masking)
                nc.vector.tensor_scalar_mul(vg[:, :, 64:65], ones16[:],
                                            maskp_sb[:, m:m + 1])
                for half in range(2):
                    ns = slice(half * 512, (half + 1) * 512)
                    p1 = pv1.tile([P, 512], F32, tag="pv1", name="pv1")
                    p2 = pv2.tile([P, 512], F32, tag="pv2", name="pv2")
                    mm(p1[:], ones2_sb[:], bv_sb[0][:, :, ns], True, False)
                    for cp in range(4):
                        mm(p1[:], xv_m[:, 2 * cp:2 * cp + 2, :],
                           wv_sb["1h"][:, 2 * cp:2 * cp + 2, ns], False,
                           (not V_SPLIT) and cp == 3)
                    if V_SPLIT:
                        for cp in range(4):
                            mm(p1[:], xv_m[:, 2 * cp:2 * cp + 2, :],
                               wv_sb["1l"][:, 2 * cp:2 * cp + 2, ns], False,
                               cp == 3)
                    mm(p2[:], ones2_sb[:], bv_sb[1][:, :, ns], True, False)
                    for cp in range(4):
                        mm(p2[:], xv_m[:, 2 * cp:2 * cp + 2, :],
                           wv_sb["2h"][:, 2 * cp:2 * cp + 2, ns], False,
                           (not V_SPLIT) and cp == 3)
                    if V_SPLIT:
                        for cp in range(4):
                            mm(p2[:], xv_m[:, 2 * cp:2 * cp + 2, :],
                               wv_sb["2l"][:, 2 * cp:2 * cp + 2, ns], False,
                               cp == 3)
                    g1v = g1vp.tile([P, 512], BF16, tag="g1v", name="g1v")
                    act(g1v[:], p1[:], ACTF.Silu, scale=1.0 / 64)
                    # vg = (silu * mask) * p2  (64-scaled; mask folded here)
                    nc.vector.scalar_tensor_tensor(
                        vg[:, 8 * half:8 * half + 8, 0:64], g1v[:],
                        maskp_sb[:, m:m + 1], p2[:], ALU.mult, ALU.mult)
                if m >= 2:
                    kv_tail(m - 2)
            kv_tail(NM - 2)
            kv_tail(NM - 1)
            kvev = [kvstage.tile([P, 260], F32, tag=f"kvev{i}", name=f"kvev{i}")
                    for i in range(2)]
            for i in range(2):
                nc.vector.tensor_copy(kvev[i][:], psum_kv[i][:])

        stv.close()  # frees wv weights
        st1.close()  # frees phi_k SBUF before stage 2

        # ============ collective: pair AllReduce of kv+ksum ============
        cc_in = dram.tile([2 * P, 260], F32)
        cc_out = dram.tile([2 * P, 260], F32)
        nc.gpsimd.dma_start(cc_in[0:P, :], kvev[0][:])
        nc.gpsimd.dma_start(cc_in[P:2 * P, :], kvev[1][:])
        kvstage_ctx.close()
        if single_core:
            nc.gpsimd.dma_start(cc_out[:], cc_in[:])
        else:
            nc.gpsimd.collective_compute(
                "AllReduce", ALU.add, replica_groups=GROUPS,
                ins=[cc_in.opt()], outs=[cc_out.opt()])

        # repack: kv -> block-diag bf16 tiles; ksum -> block-diag bf16 tiles
        kv_bd = [kvres.tile([P, P], BF16, tag=f"kvbd{c}", name=f"kvbd{c}")
                 for c in range(NCD)]
        ksum_bd = [kvres.tile([P, H], BF16, tag=f"ksbd{c}", name=f"ksbd{c}")
                   for c in range(NCD)]
        with ExitStack() as strp:
            rpp = strp.enter_context(tc.tile_pool(name="rpp", bufs=1))
            kvf32 = rpp.tile([P, NCD, DK], F32, name="kvf32")
            ksf32 = rpp.tile([P, NCD], F32, name="ksf32")
            # cc_out(row=p [+128], col=cp*65+j) is affine in (p, cp, j):
            # head h=2cp+p//64 lives at row (h//8)*128+(h%2)*64+(p%64) = p [+128]
            base = cc_out
            for lo in range(2):
                off = lo * 4 * 65 * 0 + lo * P * 260  # high half: rows 128..255
                cps = slice(4 * lo, 4 * lo + 4)
                src_kv = bass.AP(base.tensor, base.offset + off,
                                 [[260, P], [65, 4], [1, DK]])
                nc.scalar.dma_start(kvf32[:, cps, :], src_kv)
                src_ks = bass.AP(base.tensor, base.offset + off + DK,
                                 [[260, P], [65, 4], [1, 1]])
                nc.scalar.dma_start(ksf32[:, cps], src_ks)
            for cp in range(NCD):
                nc.gpsimd.memset(kv_bd[cp][:], 0.0)
                nc.vector.tensor_copy(kv_bd[cp][0:64, 0:64],
                                      kvf32[0:64, cp:cp + 1, :])
                nc.vector.tensor_copy(kv_bd[cp][64:128, 64:128],
                                      kvf32[64:128, cp:cp + 1, :])
                nc.gpsimd.memset(ksum_bd[cp][:], 0.0)
                nc.vector.tensor_copy(ksum_bd[cp][0:64, 2 * cp:2 * cp + 1],
                                      ksf32[0:64, cp:cp + 1])
                nc.vector.tensor_copy(ksum_bd[cp][64:128, 2 * cp + 1:2 * cp + 2],
                                      ksf32[64:128, cp:cp + 1])

            # ============ stage 2: q -> phi_q -> z -> attn -> out ============
            with ExitStack() as st2:
                xqp = st2.enter_context(tc.tile_pool(name="xq", bufs=3))
                g1qp = st2.enter_context(tc.tile_pool(name="g1q", bufs=2))
                qgp = st2.enter_context(tc.tile_pool(name="qg", bufs=4))
                mint2 = st2.enter_context(tc.tile_pool(name="mint2", bufs=2))
                texp2 = st2.enter_context(tc.tile_pool(name="texp2", bufs=2))
                trel2 = st2.enter_context(tc.tile_pool(name="trel2", bufs=2))
                phiqp = st2.enter_context(tc.tile_pool(name="phiq", bufs=4))
                zepsp = st2.enter_context(tc.tile_pool(name="zeps", bufs=2))
                rsbp = st2.enter_context(tc.tile_pool(name="rsb", bufs=2))
                rrepp = st2.enter_context(tc.tile_pool(name="rrep", bufs=2))
                phiqrp = st2.enter_context(tc.tile_pool(name="phiqr", bufs=2))
                rdram = st2.enter_context(tc.tile_pool(name="rdram", bufs=2,
                                                       space="DRAM"))
                attnp = st2.enter_context(tc.tile_pool(name="attn", bufs=2))
                osbp = st2.enter_context(tc.tile_pool(name="osb", bufs=2))
                pq1 = st2.enter_context(tc.tile_pool(name="pq1", bufs=2,
                                                     space="PSUM"))
                pq2 = st2.enter_context(tc.tile_pool(name="pq2", bufs=2,
                                                     space="PSUM"))
                pzp = st2.enter_context(tc.tile_pool(name="pz", bufs=1,
                                                     space="PSUM"))
                pnp = st2.enter_context(tc.tile_pool(name="pn", bufs=2,
                                                     space="PSUM"))
                pop = st2.enter_context(tc.tile_pool(name="po", bufs=1,
                                                     space="PSUM"))

                xq_tiles = {}

                def get_xq(ch):
                    if ch not in xq_tiles:
                        t = xqp.tile([P, NCD, CH], F8, tag="xq", name="xq")
                        nc.sync.dma_start(t[:], dt["xq8"][ch, :, :, :])
                        xq_tiles[ch] = t
                    return xq_tiles[ch]

                def proj(ch):
                    xq_ch = get_xq(ch)
                    if ch + 2 < NCH:
                        get_xq(ch + 2)
                    qg = qgp.tile([P, NCD, CH], BF16, tag="qg", name="qg")
                    for mc in range(NCD):
                        ms = slice(mc * P, (mc + 1) * P)
                        p1 = pq1.tile([P, CH], F32, tag="pq1", name="pq1")
                        p2 = pq2.tile([P, CH], F32, tag="pq2", name="pq2")
                        for cp in range(4):
                            mm(p1[:], wq_sb[0][:, 2 * cp:2 * cp + 2, ms],
                               xq_ch[:, 2 * cp:2 * cp + 2, :], cp == 0, cp == 3)
                        for cp in range(4):
                            mm(p2[:], wq_sb[1][:, 2 * cp:2 * cp + 2, ms],
                               xq_ch[:, 2 * cp:2 * cp + 2, :], cp == 0, cp == 3)
                        g1 = g1qp.tile([P, CH], BF16, tag="g1q", name="g1q")
                        act(g1[:], p1[:], ACTF.Silu,
                            bias=bq1c_sb[:, mc:mc + 1], scale=1.0 / 64)
                        nc.vector.scalar_tensor_tensor(
                            qg[:, mc:mc + 1, :], p2[:],
                            bq2c64_sb[:, mc:mc + 1], g1[:], ALU.add, ALU.mult)
                    return qg

                def phiq_build(qg):
                    mint = mint2.tile([P, NCD, CH], BF16, tag="mintq",
                                      name="mintq")
                    nc.vector.tensor_scalar_min(mint[:], qg[:], 0.0)
                    texp = texp2.tile([P, NCD, CH], BF16, tag="texpq",
                                      name="texpq")
                    act(texp[:], mint[:], ACTF.Exp, scale=1.0 / 64)
                    trel = trel2.tile([P, NCD, CH], BF16, tag="trelq",
                                      name="trelq")
                    nc.vector.tensor_scalar(trel[:], qg[:], 0.0, 1.0 / 64,
                                            ALU.max, ALU.mult)
                    phiq = phiqp.tile([P, NCD, CH], BF16, tag="phiq",
                                      name="phiq")
                    nc.vector.tensor_tensor(phiq[:], texp[:], trel[:], ALU.add)
                    return phiq

                def rhead(ch, phiq):
                    pz = pzp.tile([H, CH], F32, tag="pz", name="pz")
                    for cp in range(NCD):
                        nc.tensor.matmul(pz[:], ksum_bd[cp][:],
                                         phiq[:, cp:cp + 1, :],
                                         start=cp == 0, stop=cp == NCD - 1)
                    zeps = zepsp.tile([H, CH], F32, tag="zeps", name="zeps")
                    nc.vector.tensor_scalar_add(zeps[:], pz[:], EPS)
                    rsb = rsbp.tile([H, CH], F32, tag="rsb", name="rsb")
                    nc.vector.reciprocal(rsb[:], zeps[:])
                    rsb16 = rsbp.tile([H, CH], BF16, tag="rsb16", name="rsb16")
                    nc.vector.tensor_copy(rsb16[:], rsb[:])
                    # broadcast r across partitions via a DRAM round-trip: a
                    # stride-0 partition AP replicates row 2cp(+1) to 64 rows;
                    # launched a full chunk-pair before its consumers so the
                    # ~4us round trip never blocks the PE
                    rd = rdram.tile([H, CH], BF16, tag="rd", name="rd")
                    nc.sync.dma_start(rd[:], rsb16[:])
                    rrep = rrepp.tile([P, NCD, CH], BF16, tag="rrep",
                                      name="rrep")
                    base = rd[:]
                    src_lo = bass.AP(base.tensor, base.offset,
                                     [[0, 64], [2 * CH, NCD], [1, CH]])
                    src_hi = bass.AP(base.tensor, base.offset + CH,
                                     [[0, 64], [2 * CH, NCD], [1, CH]])
                    nc.sync.dma_start(rrep[0:64, :, :], src_lo)
                    nc.sync.dma_start(rrep[64:128, :, :], src_hi)
                    return rrep

                def tail2(ch, phiq, rrep):
                    # pre-scale phi_q by 1/z in ONE bf16 4x-mode DVE op, so the
                    # num matmul emits 64*attn directly and psum evicts are copies
                    phiqr = phiqrp.tile([P, NCD, CH], BF16, tag="phiqr",
                                        name="phiqr")
                    nc.vector.tensor_tensor(phiqr[:], phiq[:], rrep[:], ALU.mult)
                    attn = attnp.tile([P, NCD, CH], F8, tag="attn", name="attn")
                    for cp in range(NCD):
                        pn = pnp.tile([P, CH], F32, tag="pn", name="pn")
                        nc.tensor.matmul(pn[:], kv_bd[cp][:],
                                         phiqr[:, cp:cp + 1, :],
                                         start=True, stop=True)
                        if cp % 2 == 0:
                            nc.scalar.activation(attn[:, cp:cp + 1, :], pn[:],
                                                 ACTF.Copy)
                        else:
                            nc.vector.tensor_copy(attn[:, cp:cp + 1, :], pn[:])
                    for mt in range(2):
                        for nh in range(2):
                            ns = slice(nh * 512, (nh + 1) * 512)
                            po = pop.tile([P, 512], F32, tag="po", name="po")
                            nmm = len(wo_sb) * 4
                            i = 0
                            for wo_t in wo_sb:
                                for cp in range(4):
                                    mm(po[:],
                                       attn[:, 2 * cp:2 * cp + 2,
                                            mt * P:(mt + 1) * P],
                                       wo_t[:, 2 * cp:2 * cp + 2, ns],
                                       i == 0, i == nmm - 1)
                                    i += 1
                            osb = osbp.tile([P, 512], F32, tag="osb", name="osb")
                            if (mt + nh) % 2 == 0:
                                nc.scalar.activation(osb[:], po[:], ACTF.Copy)
                            else:
                                nc.vector.tensor_copy(osb[:], po[:])
                            row0 = ch * CH + mt * P
                            nc.gpsimd.dma_start(out[row0:row0 + P, ns], osb[:])

                get_xq(0)
                get_xq(1)
                pending = None
                for pr2 in range(NCH // 2):
                    qgs = [proj(2 * pr2), proj(2 * pr2 + 1)]
                    if pending is not None:
                        for c2, ph2, rr2 in pending:
                            tail2(c2, ph2, rr2)
                    phis = [phiq_build(qgs[i]) for i in range(2)]
                    pending = [(2 * pr2 + i, phis[i],
                                rhead(2 * pr2 + i, phis[i])) for i in range(2)]
                for c2, ph2, rr2 in pending:
                    tail2(c2, ph2, rr2)
